# revision 69
# baseline (speedup 1.0000x reference)
"""AttentionAugmentedConv2D Trainium2 kernel (8 NeuronCores, data-parallel).

Reference computation (per image):
  conv_out = conv3x3(x, conv_w) + conv_b                       [128, 32, 32]
  qkv = qkv_w @ x + qkv_b;  q*, k, v  (8 heads x 16 ch)
  logits[h] = (q_h/4)^T k_h ; w = softmax(logits); attn = v_h @ w^T
  attn = attn_w @ attn + attn_b                                [128, 32, 32]
  out = concat(conv_out, attn)                                 [256, 32, 32]

Sharding: batch 16 -> 2 images per core x 8 cores.

Per-core kernel design notes:
  * q/k are computed in fp32 but stored fp8e4m3 in a DoubleRow layout:
    channel c of head h (strip g=h%4) lives at partition 32g + c//2,
    k-half i=c%2 -- produced by even/odd-channel matmul pairs and a
    lane-local DVE bias evac.  Logits then run as fp8 DoubleRow matmuls
    (0.5 cycles/column).  The attention branch's output scale is ~560x
    below the conv branch's, so fp8 logit error is negligible globally.
  * logits computed transposed, lT[q_blk, p] = k^T q, into [128,2,512]
    PSUM tiles from a 3-buffer pool (6 banks; the rotation depth against
    the exp consumers is the throughput-limiting token count).
  * exp split across ACT (Exp, ~85 of 128 j-tiles) and GPSIMD/Pool
    (tensor_tensor pow(e,x), vpowf ucode).  GPSIMD cannot read PSUM, so
    DVE stages each Pool tile to SBUF first; ACT ~= DVE ~= 89us busy.
  * AV transposed: out[p, c] = sum_q eT[q, p] * vT[q, c], 8-matmul PSUM
    accumulation chains per (head, p-slice) with a 17-wide rhs (16 v
    channels | ones column -> softmax denominator), emitted as per-j
    blocks lagging the exps by one unit.  Output free size 17 keeps
    TensorE cost ~8x below the strip formulation.
  * per-pc epilogue (norm / transpose+evac / project) runs as three
    queue-front fillers; the transpose and projection PSUM tiles reuse
    the av-pool banks freed by the norm stage.
  * all burst work (qkv, conv taps, v transposes) is cut into small
    filler pieces consumed in fixed per-unit gap slots so PE never runs
    a long exp-independent stretch while ACT/Pool starve.
"""
import sys

sys.path.insert(0, "/opt/trn_rl_repo")
import numpy as np

import concourse.bass as bass
import concourse.mybir as mybir
import concourse.tile as tile
from concourse import bacc
from concourse.bass_utils import run_bass_kernel_spmd
from concourse.masks import make_identity

F32 = mybir.dt.float32
F32R = mybir.dt.float32r
BF16 = mybir.dt.bfloat16
FP8 = mybir.dt.float8e4
DR = mybir.MatmulPerfMode.DoubleRow
EXP = mybir.ActivationFunctionType.Exp
POW = mybir.AluOpType.pow

B, CIN, H, W = 16, 256, 32, 32
COUT, DK, DV, NH = 256, 128, 128, 8
DKH = DK // NH          # 16
CCONV = COUT - DV       # 128
HWPIX = H * W           # 1024
NCORE = 8
BPC = B // NCORE        # 2 images per core
NPC = 2                 # pixel chunks of 512
ACT_J5 = {1, 2, 4, 5, 7}      # ACT j-tiles on 5-tile units
ACT_J6 = {0, 1, 2, 4, 5, 7}   # ACT j-tiles on 6-tile units


def build():
    nc = bacc.Bacc()
    xpad_h = nc.declare_dram_parameter("xpad", [BPC, 128, 2, 34, 34], F32R, isOutput=False)
    convw_h = nc.declare_dram_parameter("convw", [9, 2, 128, 128], F32R, isOutput=False)
    qkvw_h = nc.declare_dram_parameter("qkvw", [2, 128, 9, 128], F32R, isOutput=False)
    qkb_h = nc.declare_dram_parameter("qkb", [128, 4, 2], F32, isOutput=False)
    attnw_h = nc.declare_dram_parameter("attnw", [128, 128], BF16, isOutput=False)
    bias_h = nc.declare_dram_parameter("biases", [128, 8], F32, isOutput=False)
    out_h = nc.declare_dram_parameter("out", [BPC, COUT, H, W], F32, isOutput=True)

    with tile.TileContext(nc) as tc:
        with (
            tc.tile_pool(name="singles", bufs=1) as singles,
            tc.tile_pool(name="xpadp", bufs=2) as xpadp,
            tc.tile_pool(name="qkp", bufs=2) as qkp,
            tc.tile_pool(name="vp", bufs=2) as vp,
            tc.tile_pool(name="vtp", bufs=2) as vtp,
            tc.tile_pool(name="etp", bufs=4) as etp,
            tc.tile_pool(name="nrm", bufs=2) as nrm,
            tc.tile_pool(name="outp", bufs=2) as outp,
            tc.tile_pool(name="cpp", bufs=4) as cpp,
            tc.tile_pool(name="lgps", bufs=3, space="PSUM") as lgps,
            tc.tile_pool(name="avps", bufs=2, space="PSUM") as avps,
        ):
            # ---- weights / constants to SBUF (input-critical first) ----
            qkvw = singles.tile([128, 2, 9, 128], F32R)
            qkb = singles.tile([128, 4, 2], F32)
            for ch in range(2):
                nc.sync.dma_start(out=qkvw[:, ch, :, :], in_=qkvw_h[ch, :, :, :])
            biases = singles.tile([128, 8], F32)
            convw = singles.tile([128, 9, 2, 128], F32R)
            attnw = singles.tile([128, 128], BF16)
            ident = singles.tile([128, 128], F32)
            identb = singles.tile([128, 128], BF16)
            ebase = singles.tile([128, 2, 512], F32)

            make_identity(nc, ident)
            make_identity(nc, identb)
            nc.vector.memset(ebase, float(np.e))

            def late_weights():
                nc.sync.dma_start(out=attnw, in_=attnw_h[:, :])
                nc.sync.dma_start(
                    out=convw,
                    in_=convw_h[:, :, :, :].rearrange("t c p k -> p t c k"))

            # ---------- per-image stage A pieces ----------
            stA = {}
            xp_tiles = {}

            def load_x(b):
                xp = xpadp.tile([128, 2, 34, 34], F32R, tag="xp", name=f"xp{b}")
                for half in range(2):
                    for ch in range(2):
                        nc.sync.dma_start(
                            out=xp[:, ch, 17 * half:17 * (half + 1), :],
                            in_=xpad_h[b, :, ch, 17 * half:17 * (half + 1), :])
                xp_tiles[b] = xp

            def alloc_stage_a(b):
                qa = qkp.tile([128, 2, HWPIX], FP8, tag="qa", name=f"qa{b}")
                qb = qkp.tile([128, 2, HWPIX], FP8, tag="qb", name=f"qb{b}")
                ka = qkp.tile([128, 2, HWPIX], FP8, tag="ka", name=f"ka{b}")
                kb = qkp.tile([128, 2, HWPIX], FP8, tag="kb", name=f"kb{b}")
                v_t = vp.tile([128, HWPIX], BF16, tag="v", name=f"v{b}")
                vT = vtp.tile([128, 8, 8, 17], BF16, tag="vT", name=f"vT{b}")
                stA[b] = (xp_tiles[b], qa, qb, ka, kb, vT, v_t)

            def qkv_chunk(b, ci, pcs=(0, 1)):
                """q/k group ci (qa/qb/ka/kb): per pixel chunk, even- and
                odd-channel matmuls into the two halves of an lg tile, then
                one lane-local bias evac into the fp8 DoubleRow layout."""
                xp, qa, qb, ka, kb, vT, v_t = stA[b]
                qkdst = [qa, qb, ka, kb]
                for pc in pcs:
                    tl = lgps.tile([128, 2, 512], F32, tag="lg",
                                   name=f"qk{b}_{ci}_{pc}")
                    for eo in range(2):
                        for ch in range(2):
                            nc.tensor.matmul(
                                tl[:, eo, :], qkvw[:, ch, 2 * ci + eo, :],
                                xp[:, ch, 1 + 16 * pc:17 + 16 * pc, 1:33],
                                start=(ch == 0), stop=(ch == 1))
                    nc.vector.tensor_tensor(
                        out=qkdst[ci][:, :, 512 * pc:512 * (pc + 1)],
                        in0=tl,
                        in1=qkb[:, ci, :].broadcast_to([128, 2, 512]),
                        op=mybir.AluOpType.add)

            def v_chunk(b):
                """v output: both pixel chunks + merged bias evac."""
                xp, _, _, _, _, vT, v_t = stA[b]
                tl = lgps.tile([128, 2, 512], F32, tag="lg", name=f"v{b}")
                for pc in range(NPC):
                    for ch in range(2):
                        nc.tensor.matmul(
                            tl[:, pc, :], qkvw[:, ch, 8, :],
                            xp[:, ch, 1 + 16 * pc:17 + 16 * pc, 1:33],
                            start=(ch == 0), stop=(ch == 1))
                nc.vector.tensor_scalar_add(v_t, tl, biases[:, 4:5])

            def vt_init(b):
                nc.vector.memset(stA[b][5][:, :, :, 16:17], 1.0)

            def vt_piece(b, j0):
                """Transpose v columns for j0..j0+3 and scatter into vT."""
                _, _, _, _, _, vT, v_t = stA[b]
                psb = lgps.tile([128, 2, 512], F32, tag="lg",
                                name=f"vt{b}_{j0}")[:, 0, :].bitcast(BF16)
                for t in range(4):
                    j = j0 + t
                    nc.tensor.matmul(
                        psb[:, 128 * t:128 * (t + 1)],
                        v_t[:, 128 * j:128 * (j + 1)], identb,
                        is_transpose=True, start=True, stop=True,
                        skip_group_check=True)
                nc.vector.tensor_copy(
                    vT[:, j0:j0 + 4, :, 0:16],
                    psb[:, 0:512].rearrange("p (j h c) -> p j h c", j=4, h=8))

            def conv_piece(b, pc, cell, taps):
                """A few conv tap matmuls accumulating into a held lg slice."""
                xp = stA[b][0]
                if cell[0] is None:
                    cell[0] = lgps.tile([128, 2, 512], F32, tag="lg",
                                        name=f"cv{b}_{pc}")[:, 0, :]
                slot = cell[0]
                for t in taps:
                    dy, dx = t // 3, t % 3
                    for ch in range(2):
                        nc.tensor.matmul(
                            slot,
                            convw[:, t, ch, :],
                            xp[:, ch, 16 * pc + dy:16 * pc + dy + 16, dx:dx + 32],
                            start=(t == 0 and ch == 0),
                            stop=(t == 8 and ch == 1),
                        )
                if taps[-1] == 8:
                    nc.vector.tensor_scalar_add(
                        conv_outs[b][:, 512 * pc:512 * (pc + 1)], slot,
                        biases[:, 5:6])
                    nc.sync.dma_start(
                        out=out_h[b, 0:CCONV, 16 * pc:16 * (pc + 1), :],
                        in_=conv_outs[b][:, 512 * pc:512 * (pc + 1)].rearrange(
                            "p (y x) -> p y x", y=16))

            def conv_pieces(b, pc):
                cell = [None]
                return [(lambda taps=taps: conv_piece(b, pc, cell, taps))
                        for taps in ([0, 1, 2], [3, 4], [5, 6], [7, 8])]

            # ---------- attention units ----------
            def av_block(b, pc, qpair, eTp, j):
                """The j-th matmul of all 8 accumulation chains of a unit:
                interleaved PSUM accumulation groups, one per (head, p-slice).
                Only depends on exp of j-tile j, so it can trail the exps by
                a few j positions instead of a whole unit."""
                vT = stA[b][5]
                key = (b, pc)
                if key not in avtiles:
                    avtiles[key] = (
                        avps.tile([128, 2, 256], F32, tag="av", name=f"avA{b}_{pc}"),
                        avps.tile([128, 2, 256], F32, tag="av", name=f"avB{b}_{pc}"))
                avA, avB = avtiles[key]
                for e in range(2):
                    h = 2 * qpair + e
                    for s in range(4):
                        dst = (avA if s < 2 else avB)[:, s % 2, 17 * h:17 * h + 17]
                        first_bank = (qpair == 0 and e == 0 and s % 2 == 0)
                        nc.tensor.matmul(
                            dst,
                            eTp[:, j, e, 128 * s:128 * (s + 1)],
                            vT[:, j, h, :],
                            start=(j == 0 and first_bank),
                            stop=(j == 7),
                            skip_group_check=True,
                        )

            fin_state = {}

            def finish_norm(b, pc):
                """Epilogue 1/3 (DVE only): softmax normalization."""
                avA, avB = avtiles.pop((b, pc))
                a_n = nrm.tile([128, 4, 128], BF16, tag="an")
                for t in range(2):
                    av = (avA, avB)[t]
                    rec = nrm.tile([128, 2, 8], F32, tag="rec")
                    nc.vector.reciprocal(rec, av[:, :, 16:152:17])
                    nc.vector.tensor_tensor(
                        out=a_n[:, 2 * t:2 * t + 2, :].rearrange(
                            "p s (h c) -> p s h c", h=8),
                        in0=av[:, :, 0:136].rearrange(
                            "p s (h x) -> p s h x", h=8)[:, :, :, 0:16],
                        in1=rec.broadcast_to([128, 2, 8, 16]),
                        op=mybir.AluOpType.mult)
                fin_state[(b, pc)] = a_n

            def finish_T(b, pc):
                """Epilogue 2/3: PE transposes + DVE bf16 evacuation."""
                a_n = fin_state.pop((b, pc))
                ps = avps.tile([128, 2, 256], F32, tag="av",
                               name=f"fT{b}_{pc}").rearrange(
                                   "p a b -> p (a b)").bitcast(BF16)
                rT = nrm.tile([128, 4, 128], BF16, tag="rT")
                for s in range(4):
                    nc.tensor.matmul(ps[:, 128 * s:128 * (s + 1)],
                                     a_n[:, s, :], identb,
                                     is_transpose=True, start=True, stop=True,
                                     skip_group_check=True)
                    if s % 2 == 1:
                        nc.vector.tensor_copy(
                            rT[:, s - 1:s + 1, :],
                            ps[:, 128 * (s - 1):128 * (s + 1)])
                fin_state[(b, pc)] = rT

            def finish_proj(b, pc):
                """Epilogue 3/3: projection matmuls + bias + output DMA."""
                rT = fin_state.pop((b, pc))
                po = avps.tile([128, 2, 256], F32, tag="av",
                               name=f"po{b}_{pc}").rearrange("p a b -> p (a b)")
                for s in range(4):
                    nc.tensor.matmul(
                        po[:, 128 * s:128 * (s + 1)],
                        attnw, rT[:, s, :], start=(s == 0), stop=True,
                        skip_group_check=True)
                nc.vector.tensor_scalar_add(
                    attn_outs[b][:, 512 * pc:512 * (pc + 1)], po,
                    biases[:, 6:7])
                nc.sync.dma_start(
                    out=out_h[b, CCONV:COUT, 16 * pc:16 * (pc + 1), :],
                    in_=attn_outs[b][:, 512 * pc:512 * (pc + 1)].rearrange(
                        "p (y x) -> p y x", y=16))

            # ---------- flat software pipeline ----------
            avtiles = {}
            conv_outs = {}
            attn_outs = {}
            for b in range(BPC):
                co = outp.tile([128, HWPIX], F32, tag="conv_out", name=f"co{b}")
                ao = outp.tile([128, HWPIX], F32, tag="attn_out", name=f"ao{b}")
                conv_outs[b] = co
                attn_outs[b] = ao
            units = [(b, pc, qp) for b in range(BPC) for pc in range(NPC)
                     for qp in range(4)]
            pending = []   # units whose AV is not yet emitted
            fillers = []   # queue of small burst-work closures

            unit_no = [0]

            def emit_unit(u, tail_budget=2):
                prev = pending.pop(0) if pending else None
                if u is not None:
                    idx = unit_no[0]
                    unit_no[0] += 1
                    act_set = ACT_J6 if idx < 7 else ACT_J5
                    if idx == 15:
                        act_set = {1, 3, 5, 6, 7}

                    b, pc, qp = u
                    _, qa, qb, ka, kb, _, _ = stA[b]
                    eTp = etp.tile([128, 8, 2, 512], BF16, tag="eT")
                    for j in range(8):
                        if prev is not None:
                            av_block(*prev, j)
                        if fillers and (j % 2 == 1 if prev is not None
                                        else j in (2, 4, 6)):
                            fillers.pop(0)()
                        lgp = lgps.tile([128, 2, 512], F32, tag="lg",
                                        name=f"lg{b}_{pc}_{qp}_{j}")
                        for e in range(2):
                            h = 2 * qp + e
                            g = h % 4
                            ksrc = ka if h < 4 else kb
                            qsrc = qa if h < 4 else qb
                            nc.tensor.matmul(
                                lgp[:, e, :],
                                ksrc[32 * g:32 * g + 8, :, 128 * j:128 * (j + 1)],
                                qsrc[32 * g:32 * g + 8, :, 512 * pc:512 * (pc + 1)],
                                start=True, stop=True, perf_mode=DR,
                                tile_position=(32 * g, 0),
                            )
                        if j in act_set:
                            nc.scalar.activation(eTp[:, j, :, :], lgp, EXP)
                        else:
                            # GPSIMD cannot read PSUM: stage the logits to
                            # SBUF (SP-DMA for the first two tiles per unit,
                            # DVE for the rest), Pool exps them via pow(e, x)
                            cp = cpp.tile([128, 2, 512], F32, tag="cp")
                            nc.vector.tensor_copy(cp, lgp)
                            nc.gpsimd.tensor_tensor(
                                out=eTp[:, j, :, :], in0=ebase, in1=cp, op=POW)
                    pending.append((b, pc, qp, eTp))
                else:
                    for j in range(8):
                        av_block(*prev, j)
                fin = prev is not None and prev[2] == 3
                if fin:
                    # the epilogue reuses the av-pool banks freed by its own
                    # norm stage, so it must fully run before the next pc's
                    # av blocks re-claim them (start of the next unit)
                    pb2, pp2 = prev[0], prev[1]
                    fillers.insert(0, lambda: finish_norm(pb2, pp2))
                    fillers.insert(1, lambda: finish_T(pb2, pp2))
                    fillers.insert(2, lambda: finish_proj(pb2, pp2))
                    # (order preserved; all three drain at this tail)
                for _ in range(tail_budget if u is None else (3 if fin else 1)):
                    if fillers:
                        fillers.pop(0)()

            # prologue: image 0 inputs + the q/k chunks needed by unit 0;
            # everything else becomes interleaved filler work
            load_x(0)
            alloc_stage_a(0)
            vt_init(0)
            nc.sync.dma_start(out=biases, in_=bias_h[:, :])
            nc.sync.dma_start(out=qkb, in_=qkb_h[:, :, :])
            qkv_chunk(0, 2, pcs=(0,))
            qkv_chunk(0, 0, pcs=(0,))
            if BPC > 1:
                load_x(1)
                alloc_stage_a(1)
                vt_init(1)
            late_weights()
            # deadlines: kb/qb by unit 2, v+vT by end of unit 0 (first av
            # blocks are emitted during unit 1), qa pc1 by unit 4
            fillers += [lambda: qkv_chunk(0, 2, pcs=(1,))]
            fillers += [lambda: v_chunk(0)]
            fillers += [lambda j0=j0: vt_piece(0, j0) for j0 in (0, 4)]
            fillers += [lambda ci=ci: qkv_chunk(0, ci) for ci in (3, 1)]
            fillers += [lambda: qkv_chunk(0, 0, pcs=(1,))]
            if BPC > 1:
                # image-1 stage A interleaved with image-0 conv pieces
                st1 = [lambda ci=ci: qkv_chunk(1, ci) for ci in (2, 0, 3, 1)]
                st1 += [lambda: v_chunk(1)]
                st1 += [lambda j0=j0: vt_piece(1, j0) for j0 in (0, 4)]
            else:
                st1 = []

            for u in units:
                b, pc, qp = u
                if u == (0, 0, 1):
                    fillers.extend(conv_pieces(0, 0) + conv_pieces(0, 1))
                if u == (0, 1, 0):
                    fillers.extend(st1)
                if u == (1, 0, 2) and BPC > 1:
                    fillers.extend(conv_pieces(1, 0) + conv_pieces(1, 1))
                emit_unit(u)
            while pending:
                emit_unit(None, tail_budget=8)
            while fillers:
                fillers.pop(0)()
    nc.compile()
    return nc


def _prep_inputs(x, conv_w, conv_b, qkv_w, qkv_b, attn_w, attn_b):
    """Host-side weight/layout prep shared by all cores."""
    x = np.asarray(x, np.float32)
    # padded input: [B, 2, 128, 34, 34]
    xr = x.reshape(B, 2, 128, H, W).transpose(0, 2, 1, 3, 4)
    xpad = np.zeros((B, 128, 2, H + 2, W + 2), np.float32)
    xpad[:, :, :, 1:33, 1:33] = xr

    # conv weights -> lhsT [tap, ch, cin128, cout]
    cw = np.asarray(conv_w, np.float32)            # [128, 256, 3, 3]
    convw = np.transpose(cw, (2, 3, 1, 0)).reshape(9, 2, 128, 128).copy()

    # qkv weights -> lhsT chunks [ch, cin128, 9, 128]; q/k groups are split
    # into even/odd channel slots for the fp8 DoubleRow layout:
    # channel c of head h (strip g=h%4) -> slot 2*grp + c%2, col 32g + c//2
    qw = np.asarray(qkv_w, np.float32).T           # [256, 384]
    qb_ = np.asarray(qkv_b, np.float32)
    qkvw = np.zeros((2, 128, 9, 128), np.float32)
    qkb = np.zeros((128, 4, 2), np.float32)
    biases = np.zeros((128, 8), np.float32)
    for grp, (src_base, half) in enumerate(
            ((0, 0), (0, 1), (DK, 0), (DK, 1))):   # qa qb ka kb
        scale = 0.25 if src_base == 0 else 1.0
        for g in range(4):
            for c in range(16):
                col = src_base + 64 * half + 16 * g + c
                qkvw[:, :, 2 * grp + c % 2, 32 * g + c // 2] = (
                    qw[:, col].reshape(2, 128) * scale)
                qkb[32 * g + c // 2, grp, c % 2] = qb_[col] * scale
    qkvw[:, :, 8, :] = qw[:, 2 * DK:].reshape(2, 128, 128)
    biases[:, 4] = qb_[2 * DK:]
    biases[:, 5] = np.asarray(conv_b, np.float32)
    biases[:, 6] = np.asarray(attn_b, np.float32)

    # attn projection weights, transposed [c_in(hc), c_out]
    aw = np.asarray(attn_w, np.float32)            # [128 out, 128 c]
    import ml_dtypes
    attnw = np.ascontiguousarray(aw.T).astype(ml_dtypes.bfloat16)
    return xpad, convw, qkvw, qkb, attnw, biases


_NC_CACHE = [None]


def get_nc():
    if _NC_CACHE[0] is None:
        _NC_CACHE[0] = build()
    return _NC_CACHE[0]


def run(inputs, trace=False):
    xpad, convw, qkvw, qkb, attnw, biases = _prep_inputs(**inputs)
    nc = get_nc()
    in_maps = []
    for core in range(NCORE):
        in_maps.append({
            "xpad": np.ascontiguousarray(xpad[BPC * core:BPC * (core + 1)]),
            "convw": convw, "qkvw": qkvw, "qkb": qkb, "attnw": attnw,
            "biases": biases,
        })
    res = run_bass_kernel_spmd(nc, in_maps, list(range(NCORE)), trace=trace)
    out = np.concatenate([np.asarray(res.results[i]["out"]) for i in range(NCORE)], axis=0)
    return out.astype(np.float32), res


def kernel(**inputs) -> np.ndarray:
    out, _ = run(inputs, trace=False)
    return out
